# revision 19
# baseline (speedup 1.0000x reference)
"""CapsNet forward pass on 8 Trainium2 NeuronCores (Bass/Tile).

Data-parallel: batch 256 split as 32 samples per core. Each core runs
conv1 (9x9 s1) + ReLU, primary-caps conv (9x9 s2), squash, the caps
projection (u = squash(x) @ caps_w), 3 agreement-routing iterations,
and the classification-head outputs. Weights are replicated.

Host-side prep (numpy): im2col of the input for conv1, weight layout
transposes, and a few tiny constant matrices (fold/broadcast selectors,
routing-logit init). All FLOPs that scale with the batch run on device.

Matmul dtype knob (CAPSNET_DT): fp32 | fp16 | bf16 | split16.
split16 = hi/lo fp16 decomposition: x@w ~ xh@wh + xh@wl + xl@wh, which
restores ~fp32 accuracy at 3 matmul passes (vs fp32's 4-cycle rate).

kernel(**inputs) takes the full unsharded inputs (as in
reference.setup_inputs()) and returns (y, probs) with
y: (256, 170, 8, 8) f32, probs: (256, 10) f32.
"""
import os
import numpy as np

# ---- problem constants (hardcoded) ----
NFULL = 256
NCORES = 8
NPER = 32            # samples per core
NQ = 4               # conv quarters per core
NSQ = NPER // NQ     # samples per quarter = 8
CIN = 3
HIMG = 32
K1 = 9
C1 = 256             # conv1 out channels
H1 = 24              # conv1 out spatial
S1 = H1 * H1         # 576
KI = CIN * K1 * K1   # 243 im2col rows
C2 = 256             # primary caps conv out channels (= caps dim d)
OH = 8               # conv2 out spatial
ICAPS = 64           # input caps (8x8)
NCLS = 10
ODIM = 16
MD = NCLS * ODIM     # 160
NTAP = 81
NITER = 3

DT_KNOB = os.environ.get("CAPSNET_DT", "split16")

_BUILD_CACHE = {}


def _np_dt(knob):
    if knob == "fp32":
        return np.float32
    if knob in ("fp16", "split16"):
        return np.float16
    if knob == "bf16":
        import ml_dtypes
        return ml_dtypes.bfloat16
    raise ValueError(knob)


def _nsplit(knob):
    return 2 if knob == "split16" else 1


def build(knob=DT_KNOB, reps=1, debug_taps=False):
    """Build the per-core Bass program (cached per knob/reps).

    reps>1 unrolls the whole computation multiple times in one program —
    used only for device-time measurement by wall-clock differencing.
    """
    key = (knob, reps, debug_taps)
    if key in _BUILD_CACHE:
        return _BUILD_CACHE[key]

    import concourse.bass as bass
    import concourse.bacc as bacc
    import concourse.mybir as mybir
    import concourse.tile as tile

    dtk = {"fp32": mybir.dt.float32,
           "fp16": mybir.dt.float16,
           "split16": mybir.dt.float16,
           "bf16": mybir.dt.bfloat16}[knob]
    f32 = mybir.dt.float32
    AF = mybir.ActivationFunctionType
    OP = mybir.AluOpType
    AX = mybir.AxisListType
    S = _nsplit(knob)
    split = S == 2

    nc = bacc.Bacc("TRN2", target_bir_lowering=False, debug=False,
                   num_devices=NCORES)

    # ---- DRAM I/O (per core) ----
    p_d = nc.dram_tensor("p_im2col", [S, KI, NPER * S1], dtk, kind="ExternalInput").ap()
    w1_d = nc.dram_tensor("w1t", [S, KI, C1], dtk, kind="ExternalInput").ap()
    # (s, Mc, Kc, c', tap, oc')
    w2_d = nc.dram_tensor("w2t", [S, 2, 2, 128, NTAP, 128], dtk, kind="ExternalInput").ap()
    # (s, Kc, d', caps, m)
    w3_d = nc.dram_tensor("w3t", [S, 2, 128, ICAPS, MD], dtk, kind="ExternalInput").ap()
    b1_d = nc.dram_tensor("b1t", [128, 2], f32, kind="ExternalInput").ap()
    b2_d = nc.dram_tensor("b2t", [128, 2], f32, kind="ExternalInput").ap()
    sel_d = nc.dram_tensor("sel", [128, NPER], f32, kind="ExternalInput").ap()
    sel2_d = nc.dram_tensor("sel2", [NPER, 128], f32, kind="ExternalInput").ap()
    ones_d = nc.dram_tensor("ones", [128, 1], f32, kind="ExternalInput").ap()
    binit_d = nc.dram_tensor("binit", [128, MD], f32, kind="ExternalInput").ap()
    y_d = nc.dram_tensor("y_out", [NPER, 170 * 64], f32, kind="ExternalOutput").ap()
    probs_d = nc.dram_tensor("probs_out", [NPER, NCLS], f32, kind="ExternalOutput").ap()
    l2_bounce = nc.dram_tensor("l2_bounce", [1, NPER * 64], f32, kind="Internal").ap()
    dbg = {}
    if debug_taps:
        for nm, shp, dt_ in [("d_c1", [128, 2 * NSQ * S1], dtk),
                             ("d_x", [128, 2 * NPER * 64], dtk),
                             ("d_x2", [128, NPER * 64], f32),
                             ("d_l2", [1, NPER * 64], f32),
                             ("d_fn", [NPER, ICAPS], f32),
                             ("d_un", [NPER, ICAPS * MD], f32),
                             ("d_urt", [128, 16 * MD], f32),
                             ("d_brt", [128, MD], f32)]:
            dbg[nm] = nc.dram_tensor(nm, shp, dt_, kind="ExternalOutput").ap()

    COLS = NPER * 64     # x columns: (n, oy, ox) = 2048
    QCOL = NSQ * S1      # P columns per quarter = 4608
    NCH1 = QCOL // 512   # conv1 N-chunks per quarter = 9

    with tile.TileContext(nc) as tc:
        from contextlib import ExitStack
        for _rep in range(reps):
          with ExitStack() as ctx:
            const = ctx.enter_context(tc.tile_pool(name="const", bufs=1))
            persist = ctx.enter_context(tc.tile_pool(name="persist", bufs=1))

            # constants
            w1a = [const.tile([128, C1], dtk, tag=f"w1a{s}", name=f"w1a{s}")
                   for s in range(S)]
            w1b = [const.tile([KI - 128, C1], dtk, tag=f"w1b{s}", name=f"w1b{s}")
                   for s in range(S)]
            for s in range(S):
                nc.scalar.dma_start(out=w1a[s][:], in_=w1_d[s, 0:128, :])
                nc.scalar.dma_start(out=w1b[s][:], in_=w1_d[s, 128:KI, :])
            b1_sb = const.tile([128, 2], f32, tag="b1")
            b2_sb = const.tile([128, 2], f32, tag="b2")
            nc.scalar.dma_start(out=b1_sb[:], in_=b1_d)
            nc.scalar.dma_start(out=b2_sb[:], in_=b2_d)
            sel_sb = const.tile([128, NPER], f32, tag="sel")
            sel2_sb = const.tile([NPER, 128], f32, tag="sel2")
            nc.scalar.dma_start(out=sel_sb[:], in_=sel_d)
            nc.scalar.dma_start(out=sel2_sb[:], in_=sel2_d)
            ones_sb = const.tile([128, 1], f32, tag="ones")
            nc.scalar.dma_start(out=ones_sb[:], in_=ones_d)
            b_rt = persist.tile([128, MD], f32, tag="b_rt")
            nc.scalar.dma_start(out=b_rt[:], in_=binit_d)

            # persistent activations: x[s][kc]
            x_sb = [[persist.tile([128, COLS], dtk, tag=f"x{s}{k}", name=f"x{s}{k}")
                     for k in range(2)] for s in range(S)]
            x2_sb = [persist.tile([128, COLS], f32, tag=f"x2{k}", name=f"x2{k}")
                     for k in range(2)]
            u_n = persist.tile([NPER, ICAPS * MD], f32, tag="u_n")
            u_rt = persist.tile([128, 16 * MD], f32, tag="u_rt")
            l2_sb = persist.tile([1, COLS], f32, tag="l2")
            f_n = persist.tile([NPER, ICAPS], f32, tag="f_n")

            # ---- conv phase ----
            with tc.tile_pool(name="pin", bufs=3) as pin, \
                 tc.tile_pool(name="w2p", bufs=4) as w2p, \
                 tc.tile_pool(name="c1p", bufs=1) as c1p, \
                 tc.tile_pool(name="evs", bufs=2) as evs, \
                 tc.tile_pool(name="ps1", bufs=2, space="PSUM") as ps1, \
                 tc.tile_pool(name="ps2", bufs=2, space="PSUM") as ps2:

                # c1[s][mc]
                c1 = [[c1p.tile([128, QCOL], dtk, tag=f"c1{s}{k}", name=f"c1{s}{k}")
                       for k in range(2)] for s in range(S)]

                for q in range(NQ):
                    qoff = q * QCOL
                    # conv1 for this quarter
                    for ncc in range(NCH1):
                        pa = [pin.tile([128, 512], dtk, tag=f"pa{s}", name=f"pa{s}")
                              for s in range(S)]
                        pb = [pin.tile([KI - 128, 512], dtk, tag=f"pb{s}", name=f"pb{s}")
                              for s in range(S)]
                        csl = slice(qoff + ncc * 512, qoff + (ncc + 1) * 512)
                        for s in range(S):
                            nc.scalar.dma_start(out=pa[s][:], in_=p_d[s, 0:128, csl])
                            nc.scalar.dma_start(out=pb[s][:], in_=p_d[s, 128:KI, csl])
                        # term list: (weight_s, input_s)
                        terms = [(0, 0)] if not split else [(0, 0), (1, 0), (0, 1)]
                        for mc in range(2):
                            ps = ps1.tile([128, 512], f32, tag="c1ps")
                            for ti, (ws, xs) in enumerate(terms):
                                nc.tensor.matmul(
                                    ps[:], w1a[ws][:, mc * 128:(mc + 1) * 128],
                                    pa[xs][:], start=(ti == 0), stop=False)
                                nc.tensor.matmul(
                                    ps[:], w1b[ws][:, mc * 128:(mc + 1) * 128],
                                    pb[xs][:], start=False,
                                    stop=(ti == len(terms) - 1))
                            osl = slice(ncc * 512, (ncc + 1) * 512)
                            if not split:
                                # relu(conv + bias) -> c1, cast to dtk
                                nc.scalar.activation(
                                    c1[0][mc][:, osl], ps[:],
                                    AF.Relu, bias=b1_sb[:, mc:mc + 1])
                            else:
                                cf = evs.tile([128, 512], f32, tag="c1f")
                                nc.scalar.activation(cf[:], ps[:], AF.Relu,
                                                     bias=b1_sb[:, mc:mc + 1])
                                nc.vector.tensor_copy(c1[0][mc][:, osl], cf[:])
                                nc.vector.scalar_tensor_tensor(
                                    c1[1][mc][:, osl], c1[0][mc][:, osl], -1.0,
                                    cf[:], op0=OP.mult, op1=OP.add)

                    # conv2 for this quarter: N = 8 samples * 64 = 512
                    c2terms = [(0, 0)] if not split else [(0, 0), (1, 0), (0, 1)]
                    for mc in range(2):
                        pp = ps2.tile([128, 512], f32, tag="c2ps")
                        for tap in range(NTAP):
                            dy, dx = tap // 9, tap % 9
                            if tap % 4 == 0:
                                # prefetch weights for taps [tap, tap+4)
                                ntl = min(4, NTAP - tap)
                                wcur = [[None] * 2 for _ in range(S)]
                                for s in range(S):
                                    for kc in range(2):
                                        wt = w2p.tile([128, 4 * 128], dtk,
                                                      tag=f"w2{s}{kc}",
                                                      name=f"w2{s}{kc}")
                                        nc.sync.dma_start(
                                            out=wt[:, :ntl * 128],
                                            in_=w2_d[s, mc, kc, :, tap:tap + ntl, :])
                                        wcur[s][kc] = wt
                            last_tap = (tap == NTAP - 1)
                            for ti, (ws, xs) in enumerate(c2terms):
                                for kc in range(2):
                                    rhs = c1[xs][kc][:, :].rearrange(
                                        "p (n y x) -> p n y x", n=NSQ, y=H1, x=H1)
                                    rhs = rhs[:, :, dy:dy + 2 * OH:2, dx:dx + 2 * OH:2]
                                    nc.tensor.matmul(
                                        pp[:],
                                        wcur[ws][kc][:, (tap % 4) * 128:(tap % 4 + 1) * 128],
                                        rhs,
                                        start=(tap == 0 and ti == 0 and kc == 0),
                                        stop=(last_tap and ti == len(c2terms) - 1
                                              and kc == 1))
                        xsl = slice(q * 512, (q + 1) * 512)
                        if not split:
                            # x = psum + bias (cast dtk) on DVE
                            nc.vector.tensor_scalar_add(x_sb[0][mc][:, xsl], pp[:],
                                                        b2_sb[:, mc:mc + 1])
                        else:
                            xf = evs.tile([128, 512], f32, tag="xf")
                            nc.vector.tensor_scalar_add(xf[:], pp[:],
                                                        b2_sb[:, mc:mc + 1])
                            nc.vector.tensor_copy(x_sb[0][mc][:, xsl], xf[:])
                            nc.vector.scalar_tensor_tensor(
                                x_sb[1][mc][:, xsl], x_sb[0][mc][:, xsl], -1.0,
                                xf[:], op0=OP.mult, op1=OP.add)
                        # x2 = (psum + bias)^2 on ACT
                        nc.scalar.activation(x2_sb[mc][:, xsl], pp[:],
                                             AF.Square, bias=b2_sb[:, mc:mc + 1])

            if debug_taps:
                nc.sync.dma_start(out=dbg["d_c1"][:, :NSQ * S1], in_=c1[0][0][:])
                nc.sync.dma_start(out=dbg["d_c1"][:, NSQ * S1:], in_=c1[0][1][:])
                nc.sync.dma_start(out=dbg["d_x"][:, :NPER * 64], in_=x_sb[0][0][:])
                nc.sync.dma_start(out=dbg["d_x"][:, NPER * 64:], in_=x_sb[0][1][:])
                nc.sync.dma_start(out=dbg["d_x2"], in_=x2_sb[0][:])

            # ---- caps matmuls + l2 ----
            with tc.tile_pool(name="w3p", bufs=3) as w3p, \
                 tc.tile_pool(name="psl2", bufs=2, space="PSUM") as psl2, \
                 tc.tile_pool(name="psc", bufs=3, space="PSUM") as psc:

                # l2 = sum_d x^2 : ones^T @ x2, pieces of 512 cols
                for piece in range(COLS // 512):
                    pl = psl2.tile([1, 512], f32, tag="l2ps")
                    for kc in range(2):
                        nc.tensor.matmul(
                            pl[:], ones_sb[:],
                            x2_sb[kc][:, piece * 512:(piece + 1) * 512],
                            start=(kc == 0), stop=(kc == 1))
                    nc.vector.tensor_copy(l2_sb[:, piece * 512:(piece + 1) * 512], pl[:])

                # u_raw per caps: psum [NPER, 2*MD] holds 2 caps
                cterms = [(0, 0)] if not split else [(0, 0), (0, 1), (1, 0)]
                for cg in range(ICAPS // 4):
                    w3t = [[None] * 2 for _ in range(S)]
                    for s in range(S):
                        for kc in range(2):
                            wt = w3p.tile([128, 4 * MD], dtk, tag=f"w3{s}{kc}",
                                          name=f"w3{s}{kc}")
                            nc.sync.dma_start(out=wt[:],
                                              in_=w3_d[s, kc, :, cg * 4:(cg + 1) * 4, :])
                            w3t[s][kc] = wt
                    for pair in range(2):
                        pc = psc.tile([NPER, 2 * MD], f32, tag="capsps")
                        for sub in range(2):
                            i = cg * 4 + pair * 2 + sub
                            j = pair * 2 + sub
                            nmm = len(cterms) * 2
                            mi = 0
                            for (xs, ws) in cterms:
                                for kc in range(2):
                                    lhsT = x_sb[xs][kc][:, :].rearrange(
                                        "p (n s) -> p n s", s=64)[:, :, i]
                                    nc.tensor.matmul(
                                        pc[:, sub * MD:(sub + 1) * MD],
                                        lhsT, w3t[ws][kc][:, j * MD:(j + 1) * MD],
                                        start=(mi == 0), stop=(mi == nmm - 1))
                                    mi += 1
                        # evict both caps [NPER, 2*MD] -> u_n
                        i0 = cg * 4 + pair * 2
                        eng = nc.vector if (pair % 2 == 0) else nc.scalar
                        if eng is nc.vector:
                            nc.vector.tensor_copy(
                                u_n[:, i0 * MD:(i0 + 2) * MD], pc[:])
                        else:
                            nc.scalar.activation(
                                u_n[:, i0 * MD:(i0 + 2) * MD], pc[:], AF.Copy)

            # ---- f = sqrt(l2)/(1+l2) in n-layout; scale u ----
            # f_n[n, i] <- l2_sb[0, n*64+i]; cross-partition reshape must
            # bounce through DRAM (direct 1->32 partition SBUF DMA corrupts
            # on hardware)
            nc.sync.dma_start(out=l2_bounce, in_=l2_sb[:])
            nc.sync.dma_start(
                out=f_n[:, :],
                in_=l2_bounce.rearrange("o (n i) -> (o n) i", n=NPER))
            sq = persist.tile([NPER, ICAPS], f32, tag="fsq")
            nc.scalar.activation(sq[:], f_n[:], AF.Sqrt)
            nc.vector.tensor_scalar_add(f_n[:], f_n[:], 1.0)
            nc.vector.reciprocal(f_n[:], f_n[:])
            nc.vector.tensor_mul(f_n[:], f_n[:], sq[:])
            # u_n *= f broadcast over (k,d)
            nc.vector.tensor_mul(
                u_n[:, :].rearrange("n (i m) -> n i m", m=MD),
                u_n[:, :].rearrange("n (i m) -> n i m", m=MD),
                f_n[:, :, None].broadcast_to((NPER, ICAPS, MD)))

            # build u_rt [p=(ib*32+n), (il, kd)] from u_n: per-ib partition-shifted
            # contiguous copies (DMA crosses partitions; DVE cannot)
            for ib in range(4):
                nc.sync.dma_start(
                    out=u_rt[ib * NPER:(ib + 1) * NPER, :],
                    in_=u_n[:, ib * 16 * MD:(ib + 1) * 16 * MD])

            if debug_taps:
                nc.sync.dma_start(out=dbg["d_l2"], in_=l2_sb[:])
                nc.sync.dma_start(out=dbg["d_fn"], in_=f_n[:])
                nc.sync.dma_start(out=dbg["d_un"], in_=u_n[:])
                nc.sync.dma_start(out=dbg["d_urt"], in_=u_rt[:])

            # ---- routing ----
            with tc.tile_pool(name="rt", bufs=1) as rt, \
                 tc.tile_pool(name="pss", bufs=2, space="PSUM") as pss, \
                 tc.tile_pool(name="psv", bufs=2, space="PSUM") as psv:

                tmp = rt.tile([128, 16 * MD], f32, tag="tmp")
                t_r = rt.tile([128, MD], f32, tag="t_r")
                mx = rt.tile([128, 16], f32, tag="mx")
                e_r = rt.tile([128, MD], f32, tag="e_r")
                ssum = rt.tile([128, 16], f32, tag="ssum")
                c_rt = rt.tile([128, MD], f32, tag="c_rt")
                s4 = rt.tile([128, MD], f32, tag="s4")
                v_sb = rt.tile([NPER, MD], f32, tag="v_sb")
                vrep = rt.tile([128, MD], f32, tag="vrep")
                sqs = rt.tile([NPER, MD], f32, tag="sqs")
                l2s = rt.tile([NPER, NCLS], f32, tag="l2s")
                sl = rt.tile([NPER, NCLS], f32, tag="sl")
                fs = rt.tile([NPER, NCLS], f32, tag="fs")

                for it in range(NITER + 1):
                    if it > 0:
                        # b += sum_d u*v
                        nc.vector.tensor_mul(
                            tmp[:, :].rearrange("p (il m) -> p il m", m=MD),
                            u_rt[:, :].rearrange("p (il m) -> p il m", m=MD),
                            vrep[:, None, :].broadcast_to((128, 16, MD)))
                        nc.vector.reduce_sum(
                            t_r[:, :],
                            tmp[:, :].rearrange("p (km d) -> p km d", d=ODIM),
                            axis=AX.X)
                        nc.vector.tensor_add(b_rt[:], b_rt[:], t_r[:])
                    # c = softmax(b) over k
                    bv = b_rt[:, :].rearrange("p (il k) -> p il k", k=NCLS)
                    nc.vector.reduce_max(mx[:], bv, axis=AX.X)
                    nc.vector.tensor_sub(
                        e_r[:, :].rearrange("p (il k) -> p il k", k=NCLS), bv,
                        mx[:, :, None].broadcast_to((128, 16, NCLS)))
                    nc.scalar.activation(e_r[:], e_r[:], AF.Exp)
                    nc.vector.reduce_sum(
                        ssum[:], e_r[:, :].rearrange("p (il k) -> p il k", k=NCLS),
                        axis=AX.X)
                    nc.vector.reciprocal(ssum[:], ssum[:])
                    nc.vector.tensor_mul(
                        c_rt[:, :].rearrange("p (il k) -> p il k", k=NCLS),
                        e_r[:, :].rearrange("p (il k) -> p il k", k=NCLS),
                        ssum[:, :, None].broadcast_to((128, 16, NCLS)))
                    # s = sum_i c*u  (prod, reduce over il, fold over ib)
                    nc.vector.tensor_mul(
                        tmp[:, :].rearrange("p (il k d) -> p il k d", k=NCLS, d=ODIM),
                        u_rt[:, :].rearrange("p (il k d) -> p il k d", k=NCLS, d=ODIM),
                        c_rt[:, :, None].rearrange(
                            "p (il k) o -> p il k o", k=NCLS).broadcast_to(
                            (128, 16, NCLS, ODIM)))
                    nc.vector.reduce_sum(
                        s4[:, :],
                        tmp[:, :].rearrange(
                            "p (il km) -> p km il", il=16),
                        axis=AX.X)
                    sps = pss.tile([NPER, MD], f32, tag="sps")
                    nc.tensor.matmul(sps[:], sel_sb[:], s4[:], start=True, stop=True)
                    # v = squash(s)
                    nc.scalar.activation(sqs[:], sps[:], AF.Square)
                    nc.vector.reduce_sum(
                        l2s[:], sqs[:, :].rearrange("n (k d) -> n k d", d=ODIM),
                        axis=AX.X)
                    nc.scalar.activation(sl[:], l2s[:], AF.Sqrt)
                    nc.vector.tensor_scalar_add(l2s[:], l2s[:], 1.0)
                    nc.vector.reciprocal(l2s[:], l2s[:])
                    nc.vector.tensor_mul(fs[:], sl[:], l2s[:])
                    nc.vector.tensor_mul(
                        v_sb[:, :].rearrange("n (k d) -> n k d", d=ODIM),
                        sps[:, :].rearrange("n (k d) -> n k d", d=ODIM),
                        fs[:, :, None].broadcast_to((NPER, NCLS, ODIM)))
                    if it < NITER:
                        vp = psv.tile([128, MD], f32, tag="vps")
                        nc.tensor.matmul(vp[:], sel2_sb[:], v_sb[:],
                                         start=True, stop=True)
                        nc.vector.tensor_copy(vrep[:], vp[:])

                # ---- outputs ----
                probs_sb = rt.tile([NPER, NCLS], f32, tag="probs")
                pmax = rt.tile([NPER, 1], f32, tag="pmax")
                oh_sb = rt.tile([NPER, NCLS], f32, tag="oh")
                r1 = rt.tile([NPER, NCLS * 64], f32, tag="r1")
                nc.vector.tensor_mul(sqs[:], v_sb[:], v_sb[:])
                nc.vector.reduce_sum(
                    l2s[:], sqs[:, :].rearrange("n (k d) -> n k d", d=ODIM),
                    axis=AX.X)
                nc.scalar.activation(probs_sb[:], l2s[:], AF.Sqrt)
                nc.sync.dma_start(out=probs_d, in_=probs_sb[:])
                nc.vector.reduce_max(pmax[:], probs_sb[:], axis=AX.X)
                nc.vector.tensor_scalar(oh_sb[:], probs_sb[:], pmax[:], None,
                                        op0=mybir.AluOpType.is_equal)
                nc.vector.tensor_copy(
                    r1[:, :].rearrange("n (k s) -> n k s", s=64),
                    oh_sb[:, :, None].broadcast_to((NPER, NCLS, 64)))
                nc.sync.dma_start(out=y_d[:, :NCLS * 64], in_=r1[:])
                # r2 region: scaled u in (i, m) order; host transposes to (m, i)
                nc.sync.dma_start(out=y_d[:, NCLS * 64:], in_=u_n[:])
                if debug_taps:
                    nc.sync.dma_start(out=dbg["d_brt"], in_=b_rt[:])

    nc.compile()
    _BUILD_CACHE[key] = nc
    return nc


def _split_hl(arr, knob):
    """arr (fp32) -> [S, ...] array in the kernel dtype."""
    npdt = _np_dt(knob)
    if _nsplit(knob) == 1:
        return arr.astype(npdt)[None]
    hi = arr.astype(np.float16)
    lo = (arr - hi.astype(np.float32)).astype(np.float16)
    return np.stack([hi, lo])


def prep_inputs(inputs, knob=DT_KNOB):
    """Host-side numpy prep. Returns in_maps (list of dicts, one per core)."""
    inp = np.ascontiguousarray(np.asarray(inputs["input"], dtype=np.float32))
    conv1_w = np.asarray(inputs["conv1_w"], dtype=np.float32)
    conv1_b = np.asarray(inputs["conv1_b"], dtype=np.float32)
    prim_w = np.asarray(inputs["prim_w"], dtype=np.float32)
    prim_b = np.asarray(inputs["prim_b"], dtype=np.float32)
    caps_w = np.asarray(inputs["caps_w"], dtype=np.float32)
    b_logits = np.asarray(inputs["b_logits"], dtype=np.float32)

    # im2col: P[(c,dy,dx), n*576 + y*24 + x] = input[n, c, y+dy, x+dx]
    s = inp.strides
    v = np.lib.stride_tricks.as_strided(
        inp, (NFULL, CIN, K1, K1, H1, H1),
        (s[0], s[1], s[2], s[3], s[2], s[3]))
    P = _split_hl(np.ascontiguousarray(v.transpose(1, 2, 3, 0, 4, 5)).reshape(
        KI, NFULL * S1), knob)

    w1t = _split_hl(conv1_w.reshape(C1, KI).T.copy(), knob)
    # (s, Mc, Kc, c', tap, oc')
    w2t = _split_hl(
        (prim_w.transpose(1, 2, 3, 0)          # (c, dy, dx, oc)
         .reshape(2, 128, NTAP, 2, 128)
         .transpose(3, 0, 1, 2, 4).copy()), knob)
    w3t = _split_hl(
        caps_w.transpose(1, 0, 2).reshape(2, 128, ICAPS, MD).copy(), knob)

    b1t = conv1_b.reshape(2, 128).T.copy().astype(np.float32)
    b2t = prim_b.reshape(2, 128).T.copy().astype(np.float32)
    sel = np.zeros((128, NPER), np.float32)
    sel[np.arange(128), np.arange(128) % NPER] = 1.0
    sel2 = sel.T.copy()
    ones = np.ones((128, 1), np.float32)
    binit = np.tile(b_logits.reshape(1, NCLS), (128, 16)).astype(np.float32)

    shared = {"w1t": w1t, "w2t": w2t, "w3t": w3t, "b1t": b1t, "b2t": b2t,
              "sel": sel, "sel2": sel2, "ones": ones, "binit": binit}
    in_maps = []
    for c in range(NCORES):
        m = dict(shared)
        m["p_im2col"] = np.ascontiguousarray(
            P[:, :, c * NPER * S1:(c + 1) * NPER * S1])
        in_maps.append(m)
    return in_maps


def assemble(results):
    """results: list of per-core out dicts -> (y, probs)."""
    yr = np.concatenate([r["y_out"] for r in results], axis=0)
    probs = np.concatenate([r["probs_out"] for r in results], axis=0)
    y = np.empty((NFULL, 170, 64), np.float32)
    y[:, :NCLS, :] = yr[:, :NCLS * 64].reshape(NFULL, NCLS, 64)
    # device wrote u[n, i, m]; reference r2 layout is [n, m, i]
    y[:, NCLS:, :] = yr[:, NCLS * 64:].reshape(
        NFULL, ICAPS, MD).transpose(0, 2, 1)
    return y.reshape(NFULL, 170, 8, 8), probs


def kernel(**inputs):
    from concourse import bass_utils
    nc = build()
    in_maps = prep_inputs(inputs)
    res = bass_utils.run_bass_kernel_spmd(nc, in_maps,
                                          core_ids=list(range(NCORES)))
    return assemble(res.results)
